# revision 84
# baseline (speedup 1.0000x reference)
"""Trainium2 Bass kernel for a GAT-style attention head (B=2, N=6144, H=256, O=128).

Math (matching the reference):
  seq_fts = seq @ W_fts.T                       [B, N, O]
  f1 = seq_fts @ f1_w + f1_b                    [B, N]
  f2 = seq_fts @ f2_w + f2_b                    [B, N]
  z[b, j, i]  = leaky_relu(f1[b, i] + f2[b, j], 0.01)
  coefs[b,j,i] = softmax_b(z)   (B=2 -> coefs[0] = sigmoid(z0 - z1), coefs[1] = 1 - coefs[0])
  vals[b, i, o] = sum_j coefs[b,j,i] * seq_fts[b, j, o]
  out = elu(vals + bias)

Strategy: shard the output-row dim i across 8 cores (768 rows each).
Every core redundantly computes fts for ALL j from a HOST-TRANSPOSED bf16
copy of seq (seqT[b,kt,h,j]), so no PE transposes or psum->sbuf transpose
copies are needed and HBM traffic is halved. The projection streams a
129-wide bf16 rhs (128 fts cols + g2; cols 129:257 hold g1 replicated 128x
so the f1 matmul's output lands pre-broadcast across psum partitions). The
2xNxN logits are never materialized: a fused custom DVE op computes
d = lrelu(f1_0[i]+f2_0[j]) - lrelu(f1_1[i]+f2_1[j]) per [128j x 768i] tile
(reading the f2 column straight out of the projection psum — DVE may read
psum, GPSIMD may NOT), ACT computes c0 = sigmoid(d), and the PE contracts
in a TRANSPOSED orientation: vals^T[o, i] += fts[j, o]^T @ c0[j, i] as two
wide (512/256) f32r matmuls per batch, so stage B is 4 matmuls per j-tile
instead of 7 and the output is produced as vals^T (un-transposed on the
host). vals1 uses the complement trick vals1 = colsum(fts1) - c0 @ fts1
with the colsum accumulated into spare psum columns. The mandatory
psum->sbuf fts copies alternate between ACT and DVE per j-tile, balancing
the two engines that are allowed to touch psum; the steady-state slot cost
is then max(DVE: d+copy/2, ACT: sigmoid+copy/2) ~= 1.05us over 48 j-tiles.

Per-slot engine budget (cost-model ns): PE 857 (4 proj + 4 stage-B + s1
matmuls), DVE 860 (d) + 196 (copy/2), ACT 825 (sigmoid) + 199 (copy/2),
Pool ~0 (barred from psum), DMA ~290 amortized.
"""

import numpy as np

import concourse.bacc as bacc
import concourse.bass as bass
import concourse.mybir as mybir
import concourse.tile as tile
from concourse.bass_utils import run_bass_kernel_spmd

B, N, H, O = 2, 6144, 256, 128
NCORES = 8
NS = N // NCORES          # 768 i-rows per core
NJT = N // 128            # 48 j-tiles
NIC = NS // 128           # 6 i-chunks per core
NKT = H // 128            # 2 contraction tiles
FP32 = mybir.dt.float32
BF16 = mybir.dt.bfloat16
F32R = mybir.dt.float32r
AF = mybir.ActivationFunctionType
ALU = mybir.AluOpType

# stage-B output column split: 768 = 512 + 256, each within one psum bank
HSP = ((0, 512), (512, NS))

_DVE_OP_NAME = "DIFF_LRELU_ANT"

DEFAULT_CFG = dict(
    lag=9,               # stage B lags stage A by this many j-tiles; also
                         # covers the startup latency of the f1/d/sigma chain
    bufs_fts=11,         # must exceed lag (stage_b frees fts buffers)
    bufs_d=3,
    bufs_c=6,            # c0 PAIR buffers; sp*bufs_c must exceed lag + sp
    bufs_psF=3,
    spair=1,             # j-tiles per sigmoid instruction
    csplit=-1,           # fts copy engine: -1 = alternate ACT/DVE per j-tile
)


def _get_diff_lrelu_op():
    """Register (once) and return the fused custom DVE op:
    out = lrelu(in0 + s0) - lrelu(in1 + s1), slope imm2."""
    import concourse.dve_ops as dve_ops
    from concourse.dve_ops import OPS, DveOp

    for op in OPS:
        if op.name == _DVE_OP_NAME:
            return op

    from concourse.dve_spec import C0, C1, C2, Spec, Src0, Src1, lower, maxx
    from concourse.dve_uop import DveOpSpec

    a = Src0 + C0
    b = Src1 + C1
    spec = Spec(
        body=maxx(a, a * C2) - maxx(b, b * C2),
        reference=lambda in0, in1, s0, s1, imm2: (
            np.maximum(in0 + s0, (in0 + s0) * imm2)
            - np.maximum(in1 + s1, (in1 + s1) * imm2)
        ).astype(np.float32),
    )
    row = dve_ops._CUSTOM_DVE_ROW_BASE + len(OPS)
    shas = {}
    for ver in ("v3",):
        uops = lower(spec, ver=ver)
        shas[ver] = DveOpSpec(
            name=_DVE_OP_NAME, opcode=row, uops=uops, rd1_en=True
        ).sha(ver)
    op = DveOp(_DVE_OP_NAME, spec, subdim=False, uops_sha=shas)
    OPS.append(op)
    dve_ops.CUSTOM_DVE_SPECS[_DVE_OP_NAME] = spec
    dve_ops._SUB_OPCODE_FOR_NAME[_DVE_OP_NAME] = row
    return op


def build_nc(cfg=None):
    cfg = {**DEFAULT_CFG, **(cfg or {})}
    diff_lrelu = _get_diff_lrelu_op()

    nc = bacc.Bacc("TRN2", target_bir_lowering=False, debug=False, num_devices=NCORES)

    seqT_d = nc.declare_dram_parameter("seqT", [B, NKT, 128, N], BF16, isOutput=False)
    own_d = nc.declare_dram_parameter("own", [B, NKT, 128, NS], BF16, isOutput=False)
    # wtg cols: 0:128 = W^T, 128 = g2, 129:257 = g1 replicated 128x (so the
    # f1 matmul output lands already broadcast across psum partitions)
    wtg_d = nc.declare_dram_parameter("wtg", [NKT, 128, 257], BF16, isOutput=False)
    consts_d = nc.declare_dram_parameter("consts", [1, 4], FP32, isOutput=False)
    # transposed output: vals^T, host un-transposes
    out_d = nc.declare_dram_parameter("out", [B, O, NS], FP32, isOutput=True)

    with tile.TileContext(nc) as tc:
        with (
            tc.tile_pool(name="const", bufs=1) as cpool,
            tc.tile_pool(name="fts", bufs=cfg["bufs_fts"]) as p_fts,
            tc.tile_pool(name="dtile", bufs=cfg["bufs_d"]) as p_d,
            tc.tile_pool(name="ctile", bufs=cfg["bufs_c"]) as p_c,
            tc.tile_pool(name="fin", bufs=1) as p_fin,
        ):
            # ---------------- constants / inputs ----------------
            # DMA issue order matters: the f1 chain (wt+consts+ownT) and the
            # first small seq chunks go first so PE can start ~3us in.
            wt = cpool.tile([128, NKT, 257], BF16)
            nc.sync.dma_start(wt[:], wtg_d.ap().rearrange("k h c -> h k c"))
            ownT = [cpool.tile([128, NKT, NS], BF16, name=f"ownT{b}") for b in range(B)]
            for b in range(B):
                nc.sync.dma_start(
                    ownT[b][:], own_d[b].rearrange("k h j -> h k j")
                )
            consts = cpool.tile([1, 4], FP32)
            nc.sync.dma_start(consts[:], consts_d[:])

            # full transposed seq: [h128, b, kt, j] loaded in progressively
            # larger j-chunks, each chunk ONE DMA covering all (b, kt) so the
            # serial HWDGE issue rate (625ns per DMA) doesn't gate startup
            sq = cpool.tile([128, B, NKT, N], BF16)
            lo = 0
            for sz in (512, 1024, 2048, 2560):
                nc.sync.dma_start(
                    sq[:, :, :, lo:lo + sz],
                    seqT_d[:, :, :, lo:lo + sz].rearrange("b k h j -> h b k j"),
                )
                lo += sz
            assert lo == N

            bias_col = cpool.tile([128, 1], FP32)
            nc.gpsimd.partition_broadcast(bias_col[:], consts[0:1, 1:2])
            biasm1_col = cpool.tile([128, 1], FP32)
            nc.gpsimd.partition_broadcast(biasm1_col[:], consts[0:1, 2:3])

            ones_col = cpool.tile([128, 2], FP32)
            nc.gpsimd.memset(ones_col[:], 1.0)

            f1bc2 = cpool.tile([128, B, NS], FP32)
            fsum_col = cpool.tile([128, 1], FP32)
            nc.gpsimd.partition_broadcast(fsum_col[:], consts[0:1, 0:1])

            outbufT = cpool.tile([128, B, NS], FP32)

            with (
                tc.tile_pool(name="psF", bufs=cfg["bufs_psF"], space="PSUM") as psF,
                tc.tile_pool(name="psB", bufs=1, space="PSUM") as psB,
                tc.tile_pool(name="psS", bufs=1, space="PSUM") as psS,
            ):
                # stage-B accumulators: vals^T[o, i] per b, 768 i-cols split
                # 512/256 so each matmul output stays inside one psum bank.
                # One tile per batch: the tile scheduler chains readers of a
                # tile, so separate tiles let b0/b1 finalize reads run in
                # parallel.
                pT = [
                    psB.tile([128, 2, 512], FP32, name=f"pT{b}", tag=f"pT{b}")
                    for b in range(B)
                ]
                s1T = psS.tile([128, 2], FP32, name="s1T", tag="s1T")

                # --------- f1 for own i-rows: f1[b,i] = g1 . seqT_own[:, i] ---------
                # lhsT is g1 replicated across 128 columns, so every psum
                # partition of the pT banks receives the same f1 row — the
                # broadcast is free. DVE then copies psum -> sbuf (before
                # stage B's start=True reclaims the banks). The +(f1_b+f2_b)
                # const rides on the f2 column instead (see stage_a).
                for b in range(B):
                    for h, (lo, hi) in enumerate(HSP):
                        for kt in range(NKT):
                            nc.tensor.matmul(
                                pT[b][:, h, 0:hi - lo],
                                lhsT=wt[:, kt, 129:257],
                                rhs=ownT[b][:, kt, lo:hi],
                                start=(kt == 0),
                                stop=(kt == 1),
                                skip_group_check=True,
                            )
                    # psum -> sbuf with the +(f1_b+f2_b) const folded in,
                    # emitted right after each batch's matmuls so the two
                    # chains overlap (GPSIMD cannot read psum, so b0 goes
                    # ACT Copy + Pool add, b1 goes DVE tensor_scalar)
                    if b == 0:
                        nc.scalar.activation(
                            f1bc2[:, 0, :],
                            pT[0][:].rearrange("p h c -> p (h c)")[:, 0:NS],
                            AF.Copy,
                        )
                        nc.gpsimd.tensor_scalar(
                            f1bc2[:, 0, :], f1bc2[:, 0, :], fsum_col[:],
                            None, ALU.add,
                        )
                    else:
                        nc.vector.tensor_scalar(
                            f1bc2[:, 1, :],
                            pT[1][:].rearrange("p h c -> p (h c)")[:, 0:NS],
                            fsum_col[:], None, ALU.add,
                        )

                d_tiles = {}
                fts_tiles = {}
                c0_tiles = {}

                def stage_a(jt):
                    # projection matmuls -> psum, then d (reading the f2 cols
                    # straight from psum — DVE may, GPSIMD may not), sigmoid,
                    # and finally the psum -> sbuf copies of the fts columns
                    # for stage B's lhsT (which has `lag` slots of slack).
                    # EMISSION ORDER IS LOAD-BEARING: the tile scheduler
                    # chains same-tile readers in emission order, so d must be
                    # fpp's FIRST reader and the DVE copy must precede the ACT
                    # copy, or a cross-engine d <- copy <- sigma <- d cycle
                    # forms and stretches every slot.
                    fpp = psF.tile([128, B, 129], FP32, name="fpp", tag="fpp")
                    for b in range(B):
                        for kt in range(NKT):
                            nc.tensor.matmul(
                                fpp[:, b],
                                lhsT=sq[:, b, kt, jt * 128:(jt + 1) * 128],
                                rhs=wt[:, kt, 0:129],
                                start=(kt == 0),
                                stop=(kt == 1),
                            )
                    # d tiles for SPAIR consecutive j-tiles share one buffer so
                    # ONE sigmoid instruction covers them (amortizing ACT's
                    # per-instruction access latency)
                    sp = cfg["spair"]
                    if jt % sp == 0:
                        d_tiles[jt // sp] = p_d.tile(
                            [128, sp, NS], FP32, name="d", tag="d"
                        )
                    d = d_tiles[jt // sp]
                    nc.vector._custom_dve(
                        diff_lrelu,
                        out=d[:, jt % sp, :],
                        in0=f1bc2[:, 0, :],
                        in1=f1bc2[:, 1, :],
                        s0=fpp[:, 0, 128:129],
                        s1=fpp[:, 1, 128:129],
                        imm2=0.01,
                    )
                    if jt % sp == sp - 1:
                        c0 = p_c.tile([128, sp, NS], F32R, name="c0", tag="c0")
                        nc.scalar.activation(c0[:], d_tiles.pop(jt // sp)[:], AF.Sigmoid)
                        for k in range(sp):
                            c0_tiles[jt - sp + 1 + k] = c0[:, k, :]
                    # psum -> sbuf fts copy, alternating engines per j-tile to
                    # balance DVE (d-heavy) against ACT (sigmoid-heavy)
                    # without splitting one copy across both (which costs an
                    # extra cross-engine reader-chain hop)
                    # csplit = number of j-tiles out of every 11 whose copy
                    # runs on DVE (rest on ACT); -1 = strict alternation
                    fts = p_fts.tile([128, B, 128], F32R, name="fts", tag="fts")
                    cs = cfg["csplit"]
                    on_dve = (jt % 2 == 1) if cs < 0 else (jt % 11 < cs)
                    if on_dve:
                        nc.vector.tensor_copy(fts[:], fpp[:, :, 0:128])
                    else:
                        nc.scalar.activation(fts[:], fpp[:, :, 0:128], AF.Copy)
                    fts_tiles[jt] = fts

                def stage_b(jt):
                    fts = fts_tiles.pop(jt)
                    c0 = c0_tiles.pop(jt)
                    # colsum(fts1) for the complement trick — first, so its
                    # stop (which gates the finalize's s1 columns) lands early
                    nc.tensor.matmul(
                        s1T[:], lhsT=fts[:, 1], rhs=ones_col[:].bitcast(F32R),
                        start=(jt == 0), stop=(jt == NJT - 1),
                    )
                    for b in range(B):
                        for h, (lo, hi) in enumerate(HSP):
                            nc.tensor.matmul(
                                pT[b][:, h, 0:hi - lo],
                                lhsT=fts[:, b],
                                rhs=c0[:, lo:hi],
                                start=(jt == 0),
                                stop=(jt == NJT - 1),
                                skip_group_check=True,
                            )

                lag = max(2, cfg["lag"])
                for it in range(NJT + lag):
                    if it < NJT:
                        stage_a(it)
                    if it >= lag and it - lag < NJT:
                        stage_b(it - lag)

                # ---------------- finalize: elu(vals + bias) ----------------
                # elu(y), y = vals + bias: relu(y)-1 = max(y-1,-1); + exp(min(y,0))
                # Finalize over full 768-wide contiguous psum rows (the two
                # 512-col bank halves of pT[:, b] flatten to cols 0:768).
                # b0: y = p + bias
                #   m0 = min(y, 0)           (Pool)   e0 = exp(m0)      (ACT)
                #   r0 = max(y - 1, -1)      (DVE)    out0 = r0 + e0    (DVE)
                # b1: y = s1 - p + bias   (complement trick, no x1 tile):
                #   u1 = max(p - (s1+bias), 0)  (Pool)  e1 = exp(-u1)   (ACT)
                #   t1 = min(p - (s1+bias-1), 1) (DVE)  out1 = e1 - t1  (DVE)
                p0 = pT[0][:].rearrange("p h c -> p (h c)")[:, 0:NS]
                p1 = pT[1][:].rearrange("p h c -> p (h c)")[:, 0:NS]
                s1b = p_fin.tile([128, 1], FP32, tag="fin_s1b")
                nc.vector.tensor_scalar(
                    s1b[:], s1T[:, 0:1], bias_col[:], None, ALU.add
                )
                s1bm1 = p_fin.tile([128, 1], FP32, tag="fin_s1bm1")
                nc.vector.tensor_scalar(
                    s1bm1[:], s1T[:, 0:1], biasm1_col[:], None, ALU.add
                )
                # psum readers must be DVE (GPSIMD is barred from psum);
                # sbuf-only adds go to Pool/DVE
                m0 = p_fin.tile([128, NS], FP32, tag="fin_m0")
                nc.vector.tensor_scalar(
                    m0[:], p0, bias_col[:], 0.0, ALU.add, ALU.min
                )
                r0 = p_fin.tile([128, NS], FP32, tag="fin_r0")
                nc.vector.tensor_scalar(
                    r0[:], p0, biasm1_col[:], -1.0, ALU.add, ALU.max
                )
                e0 = p_fin.tile([128, NS], FP32, tag="fin_e0")
                nc.scalar.activation(e0[:], m0[:], AF.Exp)
                nc.vector.tensor_tensor(outbufT[:, 0, :], r0[:], e0[:], ALU.add)
                nc.sync.dma_start(out_d[0], outbufT[:, 0, :])
                u1 = p_fin.tile([128, NS], FP32, tag="fin_u1")
                nc.vector.tensor_scalar(
                    u1[:], p1, s1b[:], 0.0, ALU.subtract, ALU.max
                )
                t1 = p_fin.tile([128, NS], FP32, tag="fin_t1")
                nc.vector.tensor_scalar(
                    t1[:], p1, s1bm1[:], 1.0, ALU.subtract, ALU.min
                )
                e1 = p_fin.tile([128, NS], FP32, tag="fin_e1")
                nc.scalar.activation(e1[:], u1[:], AF.Exp, scale=-1.0)
                nc.gpsimd.tensor_tensor(outbufT[:, 1, :], e1[:], t1[:], ALU.subtract)
                nc.sync.dma_start(out_d[1], outbufT[:, 1, :])

    nc.compile()
    return nc


def make_in_maps(seq, W_fts, f1_w, f1_b, f2_w, f2_b, bias):
    bf16 = mybir.dt.np(BF16)
    seq = np.asarray(seq, dtype=np.float32)
    W = np.asarray(W_fts, dtype=np.float32)
    f1_w = np.asarray(f1_w, dtype=np.float32).reshape(-1)
    f2_w = np.asarray(f2_w, dtype=np.float32).reshape(-1)
    WT = np.ascontiguousarray(W.T)                      # [H, O]
    g1 = WT @ f1_w                                      # [H]
    g2 = WT @ f2_w
    wtg = np.zeros((NKT, 128, 257), np.float32)
    for kt in range(NKT):
        wtg[kt, :, 0:O] = WT[kt * 128:(kt + 1) * 128]
        wtg[kt, :, O] = g2[kt * 128:(kt + 1) * 128]
        wtg[kt, :, O + 1:257] = g1[kt * 128:(kt + 1) * 128, None]
    wtg = wtg.astype(bf16)
    fsum = float(np.asarray(f1_b).reshape(-1)[0] + np.asarray(f2_b).reshape(-1)[0])
    bs = float(np.asarray(bias).reshape(-1)[0])
    consts = np.array([[fsum, bs, bs - 1.0, -bs]], np.float32)

    # seqT[b, kt, h, j] = seq[b, j, kt*128 + h], bf16
    seqT = np.ascontiguousarray(
        seq.transpose(0, 2, 1).reshape(B, NKT, 128, N)
    ).astype(bf16)

    in_maps = []
    for c in range(NCORES):
        in_maps.append({
            "seqT": seqT,
            "own": np.ascontiguousarray(seqT[:, :, :, c * NS:(c + 1) * NS]),
            "wtg": wtg,
            "consts": consts,
        })
    return in_maps


_NC_CACHE = []


def kernel(seq, W_fts, f1_w, f1_b, f2_w, f2_b, bias):
    if not _NC_CACHE:
        _NC_CACHE.append(build_nc())
    nc = _NC_CACHE[0]
    in_maps = make_in_maps(seq, W_fts, f1_w, f1_b, f2_w, f2_b, bias)
    res = run_bass_kernel_spmd(nc, in_maps, core_ids=list(range(NCORES)))
    # device output is vals^T ([B, O, NS] per core); un-transpose on host
    outT = np.concatenate([res.results[c]["out"] for c in range(NCORES)], axis=2)
    return np.ascontiguousarray(outT.transpose(0, 2, 1))
